# revision 14
# baseline (speedup 1.0000x reference)
"""DATMambaLayer Trainium2 kernel (Bass/Tile), data-parallel over batch B=8.

Layout strategy: everything inside each mamba runs "d-major" — channels on
SBUF partitions, sequence along the free axis — so the depthwise conv is
shifted-MACs along free, the selective scan is tensor_tensor_scan along free
(16 n-planes fused into one wide instruction with zeroed segment-start
columns), and all projections are PE matmuls. The n-reduction of C*h runs on
the PE as 16 identity-matmul accumulations into PSUM.
"""
import numpy as np
from contextlib import ExitStack

import concourse.bass as bass
import concourse.mybir as mybir
import concourse.tile as tile
from concourse.masks import make_identity

F32 = mybir.dt.float32
Alu = mybir.AluOpType
Act = mybir.ActivationFunctionType
AX = mybir.AxisListType

B, NP, D4 = 8, 196, 128
NST = 16
L1, DI1, DTR1, DM1, MT1 = 196, 1024, 32, 512, 8
L2, DI2, DTR2, DM2, MT2 = 128, 1568, 49, 784, 13  # last m2 tile has 32 valid rows
DI2P = MT2 * 128
OUT_CH = [64, 64, 128, 256]
SCALES = [16, 8, 4, 2]

INPUT_SPECS = [
    ("xs", (4, NP, D4)),
    ("w1_in", (DM1, 2 * DI1)), ("w1_x", (DI1, DTR1 + 2 * NST)),
    ("w1_dt", (DTR1, DI1)), ("w1_o", (DI1, DM1)),
    ("cv1_w", (128, MT1 * 4)), ("cv1_b", (128, MT1)), ("dt1_b", (128, MT1)),
    ("a1", (128, MT1 * NST)), ("d1", (128, MT1)),
    ("cn_g", (1, DM1)), ("cn_b", (1, DM1)),
    ("w2_in", (DM2, 2 * DI2)), ("w2_x", (DI2, DTR2 + 2 * NST)),
    ("w2_dt", (DTR2, DI2)), ("w2_o", (DI2, DM2)),
    ("cv2_w", (128, MT2 * 4)), ("cv2_b", (128, MT2)), ("dt2_b", (128, MT2)),
    ("a2", (128, MT2 * NST)), ("d2", (128, MT2)),
    ("sn_g", (1, DM2)), ("sn_b", (1, DM2)),
    ("fc1w", (4, 128, 256)), ("fc1b", (4, 128, 2)),
    ("fc2w", (4, 256, 128)), ("fc2b", (4, 128, 1)),
    ("ffg", (4, 128, 1)), ("ffb", (4, 128, 1)),
    ("rcw0", (128, 64)), ("rca0", (64, 1)), ("rcb0", (64, 1)),
    ("rcw1", (128, 64)), ("rca1", (64, 1)), ("rcb1", (64, 1)),
    ("rcw2", (128, 128)), ("rca2", (128, 1)), ("rcb2", (128, 1)),
    ("rcw3", (128, 256)), ("rca3", (256, 1)), ("rcb3", (256, 1)),
]
OUTPUT_SPECS = [
    ("o1", (64, 224, 224)), ("o2", (64, 112, 112)),
    ("o3", (128, 56, 56)), ("o4", (256, 28, 28)),
]


def _rep_ap(t, pv, nseg, ln):
    """AP that reads t[:pv, :ln] repeated nseg times along an outer free loop."""
    return bass.AP(tensor=t.tensor, offset=t.offset,
                   ap=[[t.ap[0][0], pv], [0, nseg], [1, ln]])


def emit(nc, IN, OUT, exact_gelu=True):
    with tile.TileContext(nc) as tc, ExitStack() as ctx:
        pc = ctx.enter_context(tc.tile_pool(name="pc", bufs=1))
        # PSUM budget (8 banks): mm 2 + po 2 + tps 2 + yps 2
        ps1 = ctx.enter_context(tc.tile_pool(name="ps1", bufs=2, space="PSUM"))
        ps2 = ctx.enter_context(tc.tile_pool(name="ps2", bufs=1, space="PSUM"))
        ps3 = ctx.enter_context(tc.tile_pool(name="ps3", bufs=2, space="PSUM"))
        ps4 = ctx.enter_context(tc.tile_pool(name="ps4", bufs=2, space="PSUM"))
        pdram = ctx.enter_context(tc.tile_pool(name="pdram", bufs=1, space="DRAM"))
        pscan = ctx.enter_context(tc.tile_pool(name="pscan", bufs=2))

        # ---------- constants ----------
        ident = pc.tile([128, 128], F32, name="ident", tag="ident")
        make_identity(nc, ident)
        ones_c = pc.tile([128, 1], F32, name="ones_c", tag="ones_c")
        nc.vector.memset(ones_c, 1.0)
        ones_r = pc.tile([1, 128], F32, name="ones_r", tag="ones_r")
        nc.vector.memset(ones_r, 1.0)
        eps5 = pc.tile([128, 1], F32, name="eps5", tag="eps5")
        nc.vector.memset(eps5, 1e-5)
        eps6 = pc.tile([1, 1], F32, name="eps6", tag="eps6")
        nc.vector.memset(eps6, 1e-6)

        def cload(name, tag=None):
            t = pc.tile(list(IN[name].shape), F32, tag=tag or name)
            nc.sync.dma_start(out=t, in_=IN[name])
            return t

        cv1w = cload("cv1_w"); cv1b = cload("cv1_b"); dt1b = cload("dt1_b")
        a1 = cload("a1"); d1 = cload("d1")
        cv2w = cload("cv2_w"); cv2b = cload("cv2_b"); dt2b = cload("dt2_b")
        a2 = cload("a2"); d2 = cload("d2")
        w1dt = cload("w1_dt"); w2dt = cload("w2_dt")

        cng_bc = pc.tile([128, DM1], F32, name="cng_bc", tag="cng_bc")
        nc.sync.dma_start(out=cng_bc, in_=IN["cn_g"].to_broadcast((128, DM1)))
        cnb_bc = pc.tile([128, DM1], F32, name="cnb_bc", tag="cnb_bc")
        nc.sync.dma_start(out=cnb_bc, in_=IN["cn_b"].to_broadcast((128, DM1)))
        sng_bc = pc.tile([128, DM2], F32, name="sng_bc", tag="sng_bc")
        nc.sync.dma_start(out=sng_bc, in_=IN["sn_g"].to_broadcast((128, DM2)))
        snb_bc = pc.tile([128, DM2], F32, name="snb_bc", tag="snb_bc")
        nc.sync.dma_start(out=snb_bc, in_=IN["sn_b"].to_broadcast((128, DM2)))

        # ============ PHASE A: mamba1 ============
        with tc.tile_pool(name="pa", bufs=1) as pa, \
             tc.tile_pool(name="paw", bufs=1) as paw:
            # c1 t-major tiles (raw, kept for residuals)
            ct0 = pc.tile([128, DM1], F32, name="ct0", tag="ct0")
            ct1 = pc.tile([68, DM1], F32, name="ct1", tag="ct1")
            xs_t = IN["xs"].rearrange("i t d -> t i d")
            nc.sync.dma_start(out=ct0.rearrange("t (i d) -> t i d", i=4),
                              in_=xs_t[0:128])
            nc.sync.dma_start(out=ct1.rearrange("t (i d) -> t i d", i=4),
                              in_=xs_t[128:196])

            # layernorm (t-major, feature dim = free)
            def ln_tmajor(src, pv, D, g_bc, b_bc, tg):
                stats = pa.tile([128, (D + 511) // 512, 6], F32, name=f"st{tg}", tag=f"st{tg}")
                nch = (D + 511) // 512
                csz = D // nch
                for c in range(nch):
                    nc.vector.bn_stats(out=stats[:pv, c, :],
                                       in_=src[:pv, c * csz:(c + 1) * csz])
                mv = pa.tile([128, 2], F32, name=f"mv{tg}", tag=f"mv{tg}")
                nc.vector.bn_aggr(out=mv[:pv], in_=stats[:pv])
                sd = pa.tile([128, 1], F32, name=f"sd{tg}", tag=f"sd{tg}")
                nc.scalar.activation(out=sd[:pv], in_=mv[:pv, 1:2],
                                     func=Act.Ln, bias=eps5[:pv])
                nc.scalar.activation(out=sd[:pv], in_=sd[:pv],
                                     func=Act.Exp, scale=-0.5)
                out = pa.tile([128, D], F32, name=f"cn{tg}", tag=f"cn{tg}")
                nc.vector.tensor_scalar(out=out[:pv], in0=src[:pv],
                                        scalar1=mv[:pv, 0:1], scalar2=sd[:pv],
                                        op0=Alu.subtract, op1=Alu.mult)
                nc.vector.tensor_tensor(out=out[:pv], in0=out[:pv],
                                        in1=g_bc[:pv], op=Alu.mult)
                nc.vector.tensor_tensor(out=out[:pv], in0=out[:pv],
                                        in1=b_bc[:pv], op=Alu.add)
                return out

            cn0 = ln_tmajor(ct0, 128, DM1, cng_bc, cnb_bc, "0")
            cn1 = ln_tmajor(ct1, 68, DM1, cng_bc, cnb_bc, "1")

            # transpose -> c1nT[d] [128, 196]
            c1nT = []
            for d in range(4):
                t = pa.tile([128, L1], F32, name=f"c1nT{d}", tag=f"c1nT{d}")
                p0 = ps3.tile([128, 128], F32, name="tps", tag="tps")
                nc.tensor.transpose(p0, cn0[:, 128 * d:128 * (d + 1)], ident)
                nc.scalar.copy(out=t[:, 0:128], in_=p0)
                p1 = ps3.tile([128, 68], F32, name="tps", tag="tps")
                nc.tensor.transpose(p1, cn1[0:68, 128 * d:128 * (d + 1)],
                                    ident[0:68, 0:68])
                nc.scalar.copy(out=t[:, 128:196], in_=p1)
                c1nT.append(t)

            # in_proj
            w1in = [paw.tile([128, 2 * DI1], F32, name=f"w1in{k}", tag=f"w1in{k}") for k in range(4)]
            for k in range(4):
                nc.sync.dma_start(out=w1in[k], in_=IN["w1_in"][128 * k:128 * (k + 1), :])
            xmp, zs = [], []
            for m in range(16):
                pz = ps1.tile([128, L1], F32, name="mm", tag="mm")
                for k in range(4):
                    nc.tensor.matmul(pz, w1in[k][:, 128 * m:128 * (m + 1)], c1nT[k],
                                     start=(k == 0), stop=(k == 3))
                if m < MT1:
                    t = pa.tile([128, 3 + L1], F32, name=f"xmp{m}", tag=f"xmp{m}")
                    nc.vector.memset(t[:, 0:3], 0.0)
                    nc.scalar.copy(out=t[:, 3:], in_=pz)
                    xmp.append(t)
                else:
                    zr = pa.tile([128, L1], F32, name="zraw", tag="zraw")
                    nc.scalar.copy(out=zr, in_=pz)
                    sg = pa.tile([128, L1], F32, name="zsg", tag="zsg")
                    nc.scalar.activation(out=sg, in_=pz, func=Act.Sigmoid)
                    t = pa.tile([128, L1], F32, name=f"zs{m - MT1}", tag=f"zs{m - MT1}")
                    nc.vector.tensor_tensor(out=t, in0=zr, in1=sg, op=Alu.mult)
                    zs.append(t)

            # depthwise causal conv + silu
            xms = []
            for m in range(MT1):
                cv = pa.tile([128, L1], F32, name="cv", tag="cv")
                nc.vector.tensor_scalar(out=cv, in0=xmp[m][:, 3:3 + L1],
                                        scalar1=cv1w[:, 4 * m + 3:4 * m + 4],
                                        scalar2=None, op0=Alu.mult)
                for k in range(3):
                    nc.vector.scalar_tensor_tensor(
                        out=cv, in0=xmp[m][:, k:k + L1],
                        scalar=cv1w[:, 4 * m + k:4 * m + k + 1], in1=cv,
                        op0=Alu.mult, op1=Alu.add)
                u = pa.tile([128, L1], F32, name="cvu", tag="cvu")
                nc.scalar.activation(out=u, in_=cv, func=Act.Identity,
                                     bias=cv1b[:, m:m + 1])
                sg = pa.tile([128, L1], F32, name="cvs", tag="cvs")
                nc.scalar.activation(out=sg, in_=u, func=Act.Sigmoid)
                t = pa.tile([128, L1], F32, name=f"xms{m}", tag=f"xms{m}")
                nc.vector.tensor_tensor(out=t, in0=u, in1=sg, op=Alu.mult)
                xms.append(t)

            # x_proj
            w1x = [paw.tile([128, DTR1 + 2 * NST], F32, name=f"w1x{k}", tag=f"w1x{k}") for k in range(8)]
            for k in range(8):
                nc.sync.dma_start(out=w1x[k], in_=IN["w1_x"][128 * k:128 * (k + 1), :])
            pxd = ps1.tile([64, L1], F32, name="mm", tag="mm")
            for k in range(8):
                nc.tensor.matmul(pxd, w1x[k], xms[k], start=(k == 0), stop=(k == 7))
            xdbl = pa.tile([64, L1], F32, name="xdbl", tag="xdbl")
            nc.scalar.copy(out=xdbl, in_=pxd)

            # bounce B/C rows through DRAM for partition-broadcast
            scr1 = pdram.tile([2 * NST, L1], F32, name="scr1", tag="scr1")
            nc.sync.dma_start(out=scr1, in_=xdbl[DTR1:DTR1 + 2 * NST, :])
            Bbc = pa.tile([128, NST * L1], F32, name="Bbc", tag="Bbc")
            Cbc = pa.tile([128, NST * L1], F32, name="Cbc", tag="Cbc")
            nc.sync.dma_start(
                out=Bbc.rearrange("p (n l) -> p n l", n=NST),
                in_=bass.AP(tensor=scr1.tensor, offset=scr1.offset,
                            ap=[[0, 128], [L1, NST], [1, L1]]))
            nc.sync.dma_start(
                out=Cbc.rearrange("p (n l) -> p n l", n=NST),
                in_=bass.AP(tensor=scr1.tensor, offset=scr1.offset + NST * L1,
                            ap=[[0, 128], [L1, NST], [1, L1]]))

            # dt_proj + scan per m-tile
            yg = []
            for m in range(MT1):
                pdt = ps1.tile([128, L1], F32, name="mm", tag="mm")
                nc.tensor.matmul(pdt, w1dt[:, 128 * m:128 * (m + 1)],
                                 xdbl[0:DTR1, :], start=True, stop=True)
                dtt = pa.tile([128, L1], F32, name="dtt", tag="dtt")
                nc.scalar.activation(out=dtt, in_=pdt, func=Act.Exp,
                                     bias=dt1b[:, m:m + 1])
                nc.scalar.activation(out=dtt, in_=dtt, func=Act.Ln, bias=1.0)
                dtu = pa.tile([128, L1], F32, name="dtu", tag="dtu")
                nc.vector.tensor_tensor(out=dtu, in0=dtt, in1=xms[m], op=Alu.mult)

                yps = ps4.tile([128, L1], F32, name="yps", tag="yps")
                for half in range(2):
                    n0 = half * 8
                    dA = pscan.tile([128, 8 * L1], F32, name="dA", tag="dA")
                    for n in range(8):
                        nc.scalar.activation(
                            out=dA[:, L1 * n:L1 * (n + 1)], in_=dtt, func=Act.Exp,
                            scale=a1[:, NST * m + n0 + n:NST * m + n0 + n + 1])
                    nc.vector.memset(
                        dA.rearrange("p (n l) -> p n l", n=8)[:, :, 0:1], 0.0)
                    dBu = pscan.tile([128, 8 * L1], F32, name="dBu", tag="dBu")
                    nc.gpsimd.tensor_tensor(
                        out=dBu.rearrange("p (n l) -> p n l", n=8),
                        in0=_rep_ap(dtu, 128, 8, L1),
                        in1=Bbc.rearrange("p (n l) -> p n l", n=NST)[:, n0:n0 + 8, :],
                        op=Alu.mult)
                    h = pscan.tile([128, 8 * L1], F32, name="h", tag="h")
                    nc.vector.tensor_tensor_scan(out=h, data0=dA, data1=dBu,
                                                 initial=0.0, op0=Alu.mult,
                                                 op1=Alu.add)
                    nc.vector.tensor_tensor(
                        out=h, in0=h,
                        in1=Cbc[:, L1 * n0:L1 * (n0 + 8)], op=Alu.mult)
                    for n in range(8):
                        nc.tensor.matmul(yps, ident, h[:, L1 * n:L1 * (n + 1)],
                                         start=(n0 + n == 0), stop=(n0 + n == 15),
                                         skip_group_check=True)
                yD = pa.tile([128, L1], F32, name="yD", tag="yD")
                nc.vector.scalar_tensor_tensor(out=yD, in0=xms[m],
                                               scalar=d1[:, m:m + 1], in1=yps,
                                               op0=Alu.mult, op1=Alu.add)
                t = pa.tile([128, L1], F32, name=f"yg{m}", tag=f"yg{m}")
                nc.vector.tensor_tensor(out=t, in0=yD, in1=zs[m], op=Alu.mult)
                yg.append(t)

            # out_proj -> c2 (d-major for mamba2: [dd, 4*196])
            w1o = [paw.tile([128, DM1], F32, name=f"w1o{k}", tag=f"w1o{k}") for k in range(8)]
            for k in range(8):
                nc.sync.dma_start(out=w1o[k], in_=IN["w1_o"][128 * k:128 * (k + 1), :])
            c2 = pc.tile([128, DM2], F32, name="c2", tag="c2")
            for dm in range(4):
                pm = ps1.tile([128, L1], F32, name="mm", tag="mm")
                for k in range(8):
                    nc.tensor.matmul(pm, w1o[k][:, 128 * dm:128 * (dm + 1)], yg[k],
                                     start=(k == 0), stop=(k == 7))
                nc.scalar.copy(out=c2[:, L1 * dm:L1 * (dm + 1)], in_=pm)

        # ============ PHASE B: mamba2 ============
        m2d = pc.tile([128, DM2], F32, name="m2d", tag="m2d")
        with tc.tile_pool(name="pb", bufs=1) as pb, \
             tc.tile_pool(name="pbw", bufs=1) as pbw, \
             tc.tile_pool(name="pbw2", bufs=3) as pbw2:
            # LN over features (free axis)
            stats = pb.tile([128, 2, 6], F32, name="st2", tag="st2")
            nc.vector.bn_stats(out=stats[:, 0, :], in_=c2[:, 0:392])
            nc.vector.bn_stats(out=stats[:, 1, :], in_=c2[:, 392:784])
            mv = pb.tile([128, 2], F32, name="mv2", tag="mv2")
            nc.vector.bn_aggr(out=mv, in_=stats)
            sd = pb.tile([128, 1], F32, name="sd2", tag="sd2")
            nc.scalar.activation(out=sd, in_=mv[:, 1:2], func=Act.Ln, bias=eps5)
            nc.scalar.activation(out=sd, in_=sd, func=Act.Exp, scale=-0.5)
            c2n = pb.tile([128, DM2], F32, name="c2n", tag="c2n")
            nc.vector.tensor_scalar(out=c2n, in0=c2, scalar1=mv[:, 0:1],
                                    scalar2=sd, op0=Alu.subtract, op1=Alu.mult)
            nc.vector.tensor_tensor(out=c2n, in0=c2n, in1=sng_bc, op=Alu.mult)
            nc.vector.tensor_tensor(out=c2n, in0=c2n, in1=snb_bc, op=Alu.add)

            # transpose -> c2nT[k]
            KV2 = [128] * 6 + [16]
            c2nT = []
            for k in range(7):
                kv = KV2[k]
                t = pb.tile([kv, L2], F32, name=f"c2nT{k}", tag=f"c2nT{k}")
                p0 = ps3.tile([kv, 128], F32, name="tps", tag="tps")
                nc.tensor.transpose(p0, c2n[:, 128 * k:128 * k + kv], ident)
                nc.scalar.copy(out=t, in_=p0)
                c2nT.append(t)

            # in_proj2: resident halves of w2_in (xm cols then z cols)
            pvm = [128] * 12 + [32]

            def inproj2_wave(col0, dst_cb):
                w2in = [pbw.tile([KV2[k], DI2], F32, name=f"w2in{k}", tag=f"w2in{k}") for k in range(7)]
                for k in range(7):
                    nc.sync.dma_start(
                        out=w2in[k],
                        in_=IN["w2_in"][128 * k:128 * k + KV2[k], col0:col0 + DI2])
                for m in range(MT2):
                    pv = pvm[m]
                    pz = ps1.tile([pv, L2], F32, name="mm", tag="mm")
                    for k in range(7):
                        nc.tensor.matmul(pz, w2in[k][:, 128 * m:128 * m + pv],
                                         c2nT[k], start=(k == 0), stop=(k == 6))
                    dst_cb(m, pv, pz)

            xmp2, zs2 = [], []
            for m in range(MT2):
                xmp2.append(pb.tile([128, 3 + L2], F32, name=f"xmp2_{m}", tag=f"xmp2_{m}"))
                zs2.append(pb.tile([128, L2], F32, name=f"zs2_{m}", tag=f"zs2_{m}"))

            def xm_cb(m, pv, pz):
                nc.vector.memset(xmp2[m][:pv, 0:3], 0.0)
                nc.scalar.copy(out=xmp2[m][:pv, 3:], in_=pz)

            def z_cb(m, pv, pz):
                zr = pb.tile([128, L2], F32, name="zraw2", tag="zraw2")
                nc.scalar.copy(out=zr[:pv], in_=pz)
                sg = pb.tile([128, L2], F32, name="zsg2", tag="zsg2")
                nc.scalar.activation(out=sg[:pv], in_=pz, func=Act.Sigmoid)
                nc.vector.tensor_tensor(out=zs2[m][:pv], in0=zr[:pv],
                                        in1=sg[:pv], op=Alu.mult)

            inproj2_wave(0, xm_cb)

            # conv + silu
            xms2 = []
            for m in range(MT2):
                pv = pvm[m]
                cv = pb.tile([128, L2], F32, name="cv2", tag="cv2")
                nc.vector.tensor_scalar(out=cv[:pv], in0=xmp2[m][:pv, 3:3 + L2],
                                        scalar1=cv2w[:pv, 4 * m + 3:4 * m + 4],
                                        scalar2=None, op0=Alu.mult)
                for k in range(3):
                    nc.vector.scalar_tensor_tensor(
                        out=cv[:pv], in0=xmp2[m][:pv, k:k + L2],
                        scalar=cv2w[:pv, 4 * m + k:4 * m + k + 1], in1=cv[:pv],
                        op0=Alu.mult, op1=Alu.add)
                u = pb.tile([128, L2], F32, name="cvu2", tag="cvu2")
                nc.scalar.activation(out=u[:pv], in_=cv[:pv], func=Act.Identity,
                                     bias=cv2b[:pv, m:m + 1])
                sg = pb.tile([128, L2], F32, name="cvs2", tag="cvs2")
                nc.scalar.activation(out=sg[:pv], in_=u[:pv], func=Act.Sigmoid)
                t = pb.tile([128, L2], F32, name=f"xms2_{m}", tag=f"xms2_{m}")
                nc.vector.tensor_tensor(out=t[:pv], in0=u[:pv], in1=sg[:pv],
                                        op=Alu.mult)
                xms2.append(t)

            # x_proj2
            w2x = [pbw.tile([128, DTR2 + 2 * NST], F32, name=f"w2x{k}", tag=f"w2x{k}")
                   for k in range(MT2)]
            for k in range(MT2):
                nc.sync.dma_start(out=w2x[k][:pvm[k]],
                                  in_=IN["w2_x"][128 * k:128 * k + pvm[k], :])
            pxd2 = ps1.tile([DTR2 + 2 * NST, L2], F32, name="mm", tag="mm")
            for k in range(MT2):
                pv = pvm[k]
                nc.tensor.matmul(pxd2, w2x[k][:pv], xms2[k][:pv],
                                 start=(k == 0), stop=(k == MT2 - 1))
            xdbl2 = pb.tile([DTR2 + 2 * NST, L2], F32, name="xdbl2", tag="xdbl2")
            nc.scalar.copy(out=xdbl2, in_=pxd2)

            scr2 = pdram.tile([2 * NST, L2], F32, name="scr2", tag="scr2")
            nc.sync.dma_start(out=scr2, in_=xdbl2[DTR2:DTR2 + 2 * NST, :])
            Bbc2 = pb.tile([128, NST * L2], F32, name="Bbc2", tag="Bbc2")
            Cbc2 = pb.tile([128, NST * L2], F32, name="Cbc2", tag="Cbc2")
            nc.sync.dma_start(
                out=Bbc2.rearrange("p (n l) -> p n l", n=NST),
                in_=bass.AP(tensor=scr2.tensor, offset=scr2.offset,
                            ap=[[0, 128], [L2, NST], [1, L2]]))
            nc.sync.dma_start(
                out=Cbc2.rearrange("p (n l) -> p n l", n=NST),
                in_=bass.AP(tensor=scr2.tensor, offset=scr2.offset + NST * L2,
                            ap=[[0, 128], [L2, NST], [1, L2]]))

            # z-half of in_proj2 (overlaps with scan work below)
            inproj2_wave(DI2, z_cb)

            # dt_proj + scan
            yg2 = []
            for m in range(MT2):
                pv = pvm[m]
                pdt = ps1.tile([pv, L2], F32, name="mm", tag="mm")
                nc.tensor.matmul(pdt, w2dt[:, 128 * m:128 * m + pv],
                                 xdbl2[0:DTR2, :], start=True, stop=True)
                dtt = pb.tile([128, L2], F32, name="dtt2", tag="dtt2")
                nc.scalar.activation(out=dtt[:pv], in_=pdt, func=Act.Exp,
                                     bias=dt2b[:pv, m:m + 1])
                nc.scalar.activation(out=dtt[:pv], in_=dtt[:pv], func=Act.Ln,
                                     bias=1.0)
                dtu = pb.tile([128, L2], F32, name="dtu2", tag="dtu2")
                nc.vector.tensor_tensor(out=dtu[:pv], in0=dtt[:pv],
                                        in1=xms2[m][:pv], op=Alu.mult)
                yps = ps4.tile([pv, L2], F32, name="yps", tag="yps")
                for half in range(2):
                    n0 = half * 8
                    dA = pscan.tile([128, 8 * L2], F32, name="dA", tag="dA")
                    for n in range(8):
                        nc.scalar.activation(
                            out=dA[:pv, L2 * n:L2 * (n + 1)], in_=dtt[:pv],
                            func=Act.Exp,
                            scale=a2[:pv, NST * m + n0 + n:NST * m + n0 + n + 1])
                    nc.vector.memset(
                        dA.rearrange("p (n l) -> p n l", n=8)[:pv, :, 0:1], 0.0)
                    dBu = pscan.tile([128, 8 * L2], F32, name="dBu", tag="dBu")
                    nc.gpsimd.tensor_tensor(
                        out=dBu.rearrange("p (n l) -> p n l", n=8)[:pv],
                        in0=_rep_ap(dtu, pv, 8, L2),
                        in1=Bbc2.rearrange("p (n l) -> p n l", n=NST)[:pv, n0:n0 + 8, :],
                        op=Alu.mult)
                    h = pscan.tile([128, 8 * L2], F32, name="h", tag="h")
                    nc.vector.tensor_tensor_scan(out=h[:pv], data0=dA[:pv],
                                                 data1=dBu[:pv], initial=0.0,
                                                 op0=Alu.mult, op1=Alu.add)
                    nc.vector.tensor_tensor(out=h[:pv], in0=h[:pv],
                                            in1=Cbc2[:pv, L2 * n0:L2 * (n0 + 8)],
                                            op=Alu.mult)
                    for n in range(8):
                        nc.tensor.matmul(yps, ident[:pv, :pv],
                                         h[:pv, L2 * n:L2 * (n + 1)],
                                         start=(n0 + n == 0), stop=(n0 + n == 15),
                                         skip_group_check=True)
                yD = pb.tile([128, L2], F32, name="yD2", tag="yD2")
                nc.vector.scalar_tensor_tensor(out=yD[:pv], in0=xms2[m][:pv],
                                               scalar=d2[:pv, m:m + 1], in1=yps,
                                               op0=Alu.mult, op1=Alu.add)
                t = pb.tile([128, L2], F32, name=f"yg2_{m}", tag=f"yg2_{m}")
                nc.vector.tensor_tensor(out=t[:pv], in0=yD[:pv], in1=zs2[m][:pv],
                                        op=Alu.mult)
                yg2.append(t)

            # out_proj2 (stationary swap): out[dd, r] += yg2[k].T @ w2_o[k]
            poA = ps2.tile([128, 392], F32, name="po0", tag="po0")
            poB = ps2.tile([128, 392], F32, name="po1", tag="po1")
            for k in range(MT2):
                pv = pvm[k]
                wo = pbw2.tile([128, DM2], F32, name="w2o", tag="w2o")
                nc.sync.dma_start(out=wo[:pv], in_=IN["w2_o"][128 * k:128 * k + pv, :])
                nc.tensor.matmul(poA, yg2[k][:pv], wo[:pv, 0:392],
                                 start=(k == 0), stop=(k == MT2 - 1),
                                 skip_group_check=True)
                nc.tensor.matmul(poB, yg2[k][:pv], wo[:pv, 392:784],
                                 start=(k == 0), stop=(k == MT2 - 1),
                                 skip_group_check=True)
            nc.scalar.copy(out=m2d[:, 0:392], in_=poA)
            nc.scalar.copy(out=m2d[:, 392:784], in_=poB)

        # ============ PHASE C: residual + MLP + recon + upsample ============
        with tc.tile_pool(name="pcc", bufs=1) as pcc, \
             tc.tile_pool(name="pup", bufs=2) as pup:
            for i in range(4):
                # xiT via PE transpose of raw inputs
                xiT = pcc.tile([128, L1], F32, name="xiT", tag="xiT")
                p0 = ps3.tile([128, 128], F32, name="tps", tag="tps")
                nc.tensor.transpose(p0, ct0[:, 128 * i:128 * (i + 1)], ident)
                nc.scalar.copy(out=xiT[:, 0:128], in_=p0)
                p1 = ps3.tile([128, 68], F32, name="tps", tag="tps")
                nc.tensor.transpose(p1, ct1[0:68, 128 * i:128 * (i + 1)],
                                    ident[0:68, 0:68])
                nc.scalar.copy(out=xiT[:, 128:196], in_=p1)

                riT = pcc.tile([128, L1], F32, name="riT", tag="riT")
                nc.vector.tensor_tensor(out=riT, in0=xiT,
                                        in1=m2d[:, L1 * i:L1 * (i + 1)], op=Alu.add)

                # LN over dd (partition axis) via PE-ones reductions
                sq = pcc.tile([128, L1], F32, name="sq", tag="sq")
                nc.scalar.activation(out=sq, in_=riT, func=Act.Square)
                pmu = ps3.tile([1, L1], F32, name="tps", tag="tps")
                nc.tensor.matmul(pmu, ones_c, riT, start=True, stop=True)
                pms = ps3.tile([1, L1], F32, name="tps", tag="tps")
                nc.tensor.matmul(pms, ones_c, sq, start=True, stop=True)
                mu = pcc.tile([1, L1], F32, name="mu", tag="mu")
                nc.scalar.mul(out=mu, in_=pmu, mul=1.0 / 128)
                ms = pcc.tile([1, L1], F32, name="ms", tag="ms")
                nc.scalar.mul(out=ms, in_=pms, mul=1.0 / 128)
                var = pcc.tile([1, L1], F32, name="var", tag="var")
                nc.vector.tensor_tensor(out=var, in0=mu, in1=mu, op=Alu.mult)
                nc.vector.tensor_tensor(out=var, in0=ms, in1=var, op=Alu.subtract)
                nc.scalar.activation(out=var, in_=var, func=Act.Ln, bias=eps6)
                nc.scalar.activation(out=var, in_=var, func=Act.Exp, scale=-0.5)
                pmub = ps3.tile([128, L1], F32, name="tps", tag="tps")
                nc.tensor.matmul(pmub, ones_r, mu, start=True, stop=True)
                prsb = ps3.tile([128, L1], F32, name="tps", tag="tps")
                nc.tensor.matmul(prsb, ones_r, var, start=True, stop=True)
                rn = pcc.tile([128, L1], F32, name="rn", tag="rn")
                nc.vector.tensor_tensor(out=rn, in0=riT, in1=pmub, op=Alu.subtract)
                nc.vector.tensor_tensor(out=rn, in0=rn, in1=prsb, op=Alu.mult)
                ffg_t = pcc.tile([128, 1], F32, name="ffg_t", tag="ffg_t")
                nc.sync.dma_start(out=ffg_t, in_=IN["ffg"][i])
                ffb_t = pcc.tile([128, 1], F32, name="ffb_t", tag="ffb_t")
                nc.sync.dma_start(out=ffb_t, in_=IN["ffb"][i])
                nc.vector.tensor_scalar(out=rn, in0=rn, scalar1=ffg_t,
                                        scalar2=ffb_t, op0=Alu.mult, op1=Alu.add)

                # MLP
                f1w = pcc.tile([128, 256], F32, name="f1w", tag="f1w")
                nc.sync.dma_start(out=f1w, in_=IN["fc1w"][i])
                f1b = pcc.tile([128, 2], F32, name="f1b", tag="f1b")
                nc.sync.dma_start(out=f1b, in_=IN["fc1b"][i])
                f2w = pcc.tile([128, 2, 128], F32, name="f2w", tag="f2w")
                nc.sync.dma_start(out=f2w, in_=IN["fc2w"][i].rearrange(
                    "(a b) c -> b a c", b=128))
                f2b = pcc.tile([128, 1], F32, name="f2b", tag="f2b")
                nc.sync.dma_start(out=f2b, in_=IN["fc2b"][i])
                hm = []
                for mi in range(2):
                    pf = ps1.tile([128, L1], F32, name="mm", tag="mm")
                    nc.tensor.matmul(pf, f1w[:, 128 * mi:128 * (mi + 1)], rn,
                                     start=True, stop=True)
                    v = pcc.tile([128, L1], F32, name="gv", tag="gv")
                    nc.scalar.activation(out=v, in_=pf, func=Act.Identity,
                                         bias=f1b[:, mi:mi + 1])
                    ef = pcc.tile([128, L1], F32, name="gef", tag="gef")
                    if exact_gelu:
                        nc.scalar.activation(out=ef, in_=v, func=Act.Erf,
                                             scale=0.7071067811865476)
                    else:
                        sq = pcc.tile([128, L1], F32, name="gsq", tag="gsq")
                        nc.scalar.activation(out=sq, in_=v, func=Act.Square)
                        nc.vector.tensor_scalar(out=sq, in0=sq, scalar1=0.044715,
                                                scalar2=1.0, op0=Alu.mult,
                                                op1=Alu.add)
                        nc.vector.tensor_tensor(out=sq, in0=sq, in1=v, op=Alu.mult)
                        nc.scalar.activation(out=ef, in_=sq, func=Act.Tanh,
                                             scale=0.7978845608028654)
                    t = pcc.tile([128, L1], F32, name=f"hm{mi}", tag=f"hm{mi}")
                    nc.vector.scalar_tensor_tensor(out=t, in0=ef, scalar=1.0,
                                                   in1=v, op0=Alu.add,
                                                   op1=Alu.mult)
                    hm.append(t)
                pg = ps1.tile([128, L1], F32, name="mm", tag="mm")
                for k in range(2):
                    nc.tensor.matmul(pg, f2w[:, k, :], hm[k],
                                     start=(k == 0), stop=(k == 1))
                ymlp = pcc.tile([128, L1], F32, name="ymlp", tag="ymlp")
                nc.vector.scalar_tensor_tensor(out=ymlp, in0=pg, scalar=f2b,
                                               in1=riT, op0=Alu.add, op1=Alu.add)

                # recon conv + bn + relu at 14x14, then upsample
                oc = OUT_CH[i]
                s = SCALES[i]
                rcw = pcc.tile([128, oc], F32, name="rcw", tag="rcw")
                nc.sync.dma_start(out=rcw, in_=IN[f"rcw{i}"])
                bm = min(oc, 128)
                nmo = (oc + 127) // 128
                rca_t = pcc.tile([128, nmo], F32, name="rca_t", tag="rca_t")
                nc.sync.dma_start(out=rca_t[:bm], in_=IN[f"rca{i}"].rearrange(
                    "(a b) c -> b (a c)", b=bm))
                rcb_t = pcc.tile([128, nmo], F32, name="rcb_t", tag="rcb_t")
                nc.sync.dma_start(out=rcb_t[:bm], in_=IN[f"rcb{i}"].rearrange(
                    "(a b) c -> b (a c)", b=bm))
                out4 = OUT[f"o{i + 1}"].rearrange("o (h dh) w -> o h dh w", dh=s)
                for mo in range((oc + 127) // 128):
                    ov = min(128, oc - 128 * mo)
                    pr = ps1.tile([ov, L1], F32, name="mm", tag="mm")
                    nc.tensor.matmul(pr, rcw[:, 128 * mo:128 * mo + ov], ymlp,
                                     start=True, stop=True)
                    small = pcc.tile([128, L1], F32, name="small", tag="small")
                    nc.scalar.activation(out=small[:ov], in_=pr, func=Act.Relu,
                                         scale=rca_t[:ov, mo:mo + 1],
                                         bias=rcb_t[:ov, mo:mo + 1])
                    wide = pup.tile([128, 14, 14 * s], F32, name="wide", tag="wide")
                    sm3 = small.rearrange("p (h w) -> p h w", h=14)
                    for dw in range(s):
                        nc.gpsimd.tensor_copy(out=wide[:ov, :, dw::s],
                                              in_=sm3[:ov])
                    for dh in range(s):
                        nc.sync.dma_start(
                            out=out4[128 * mo:128 * mo + ov, :, dh, :],
                            in_=wide[:ov])
    return nc


# ---------------- host side ----------------

def _np(a):
    return np.asarray(a, dtype=np.float32)


def _chan_layout(v, mt):
    """(C,) or (C,k) channel-major vector -> (128, mt*k) partition-major tiles."""
    v = _np(v)
    if v.ndim == 1:
        v = v[:, None]
    k = v.shape[1]
    pad = mt * 128 - v.shape[0]
    if pad:
        v = np.concatenate([v, np.zeros((pad, k), np.float32)], 0)
    return v.reshape(mt, 128, k).transpose(1, 0, 2).reshape(128, mt * k).copy()


def _prep(params):
    p = params
    cm, sm = p["cm"], p["sm"]
    d = {}
    d["w1_in"] = _np(cm["in_proj_w"]).T.copy()
    d["w1_x"] = _np(cm["x_proj_w"]).T.copy()
    d["w1_dt"] = _np(cm["dt_proj_w"]).T.copy()
    d["w1_o"] = _np(cm["out_proj_w"]).T.copy()
    d["cv1_w"] = _chan_layout(cm["conv_w"], MT1)
    d["cv1_b"] = _chan_layout(cm["conv_b"], MT1)
    d["dt1_b"] = _chan_layout(cm["dt_proj_b"], MT1)
    d["a1"] = _chan_layout(-np.exp(_np(cm["A_log"])), MT1)
    d["d1"] = _chan_layout(cm["D"], MT1)
    d["cn_g"] = _np(p["cnorm_g"])[None, :]
    d["cn_b"] = _np(p["cnorm_b"])[None, :]

    def pad2(w):
        w = _np(w)
        out = np.zeros((DI2P, w.shape[1]), np.float32)
        out[:DI2] = w
        return out
    d["w2_in"] = _np(sm["in_proj_w"]).T.copy()
    d["w2_x"] = _np(sm["x_proj_w"]).T.copy()
    d["w2_dt"] = _np(sm["dt_proj_w"]).T.copy()
    d["w2_o"] = _np(sm["out_proj_w"]).T.copy()
    d["cv2_w"] = _chan_layout(pad2(_np(sm["conv_w"])), MT2)
    d["cv2_b"] = _chan_layout(pad2(_np(sm["conv_b"])[:, None]), MT2)
    d["dt2_b"] = _chan_layout(pad2(_np(sm["dt_proj_b"])[:, None]), MT2)
    a2v = -np.exp(_np(sm["A_log"]))
    a2p = np.full((DI2P, NST), -1.0, np.float32)
    a2p[:DI2] = a2v
    d["a2"] = _chan_layout(a2p, MT2)
    d["d2"] = _chan_layout(pad2(_np(sm["D"])[:, None]), MT2)
    d["sn_g"] = _np(p["snorm_g"])[None, :]
    d["sn_b"] = _np(p["snorm_b"])[None, :]

    d["fc1w"] = np.stack([_np(f["fc1_w"]).T for f in p["ffn"]])
    d["fc1b"] = np.stack([_np(f["fc1_b"]).reshape(2, 128).T.copy()
                          for f in p["ffn"]])
    d["fc2w"] = np.stack([0.5 * _np(f["fc2_w"]).T for f in p["ffn"]])
    d["fc2b"] = np.stack([_np(f["fc2_b"])[:, None] for f in p["ffn"]])
    d["ffg"] = _np(p["ffn_g"])[:, :, None].copy()
    d["ffb"] = _np(p["ffn_b"])[:, :, None].copy()

    for i, r in enumerate(p["recon"]):
        alpha = _np(r["bn_g"]) / np.sqrt(_np(r["bn_rv"]) + 1e-5)
        beta = (_np(r["conv_b"]) - _np(r["bn_rm"])) * alpha + _np(r["bn_b"])
        d[f"rcw{i}"] = _np(r["conv_w"]).T.copy()
        d[f"rca{i}"] = alpha[:, None].copy()
        d[f"rcb{i}"] = beta[:, None].copy()
    return d


_CACHE = {}


def _get_program():
    if "nc" not in _CACHE:
        from concourse import bacc
        nc = bacc.Bacc("TRN2")
        IN = {}
        for name, shape in INPUT_SPECS:
            IN[name] = nc.dram_tensor(name, list(shape), F32,
                                      kind="ExternalInput").ap()
        OUT = {}
        for name, shape in OUTPUT_SPECS:
            OUT[name] = nc.dram_tensor(name, list(shape), F32,
                                       kind="ExternalOutput").ap()
        emit(nc, IN, OUT)
        nc.compile()
        _CACHE["nc"] = nc
    return _CACHE["nc"]


def kernel(x1, x2, x3, x4, params):
    from concourse.bass_utils import run_bass_kernel_spmd
    nc = _get_program()
    w = _prep(params)
    xs = np.stack([_np(x1), _np(x2), _np(x3), _np(x4)], axis=1)  # (B, 4, NP, D4)
    in_maps = []
    for b in range(B):
        m = dict(w)
        m["xs"] = np.ascontiguousarray(xs[b])
        in_maps.append(m)
    res = run_bass_kernel_spmd(nc, in_maps, core_ids=list(range(B)))
    outs = []
    for name, shape in OUTPUT_SPECS:
        outs.append(np.stack([res.results[b][name] for b in range(B)], axis=0))
    return tuple(outs)
